# revision 16
# baseline (speedup 1.0000x reference)
"""Trainium2 Bass kernel for nn_AttentionModel_88905823027207.

Full inputs:  x [4, 2048, 1024] f32, w_qkv [1024, 3072] f32, w_out [1024, 1024] f32
Full output:  [4, 2048, 1024] f32  (multi-head attention, 16 heads, + out proj)

Sharding: 8 cores = (batch b in 0..3) x (head-group g in 0..1).
Each core computes 8 heads of one batch element and the partial out-projection
for its head-group's rows of w_out; the host sums the two partials per batch.

Per-core kernel (all matmuls bf16 with fp32 PSUM accumulation):
  stage1: qT,kT [512, S] = w{q,k}.T @ x.T (dt-outer so each LDWEIGHTS serves
          4 matmuls); vhat [S, 8*65] = x @ wv with a ones-column per head
          (attnV then also accumulates the softmax denominator as row 64).
  attention (per head-pair, row-packed across PE row-groups via partition
          offsets 0/64): scoresT [k, q] -> exp(0.125*s) -> bf16, split
          between the Scalar engine (ACT spline exp) and the Vector engine
          (Schraudolph int16 bit-trick exp) so neither is the bottleneck
          -> attnV accumulation [65, 512] in PSUM (double-buffered).
  normalize: denom rows -> bf16 on Scalar, PE ones-matmul broadcast, DVE
          reciprocal_approx_fast, fused (PSUM evac x recip) multiplies.
  out-proj: attn_normT @ w_out rows, Scalar-engine evacuation, fp32 partial
          out to DRAM.

PSUM budget: one shared [128,1024] ring (bufs=2, 4 banks) carries scores /
stage1 accumulators / broadcast / out-proj; attnV pair is double-buffered
(4 banks). Total 8 banks.
"""

import numpy as np
import ml_dtypes

BF16 = ml_dtypes.bfloat16

# Full-problem dims (hardcoded per harness contract)
B_FULL, S_FULL, D_FULL, H_FULL, HD = 4, 2048, 1024, 16, 64
N_CORES = 8
HEADS_PER_CORE = H_FULL // 2  # 8

# Schraudolph bf16 exp constants: int16(trunc(s*0.125*184.665 + 16250.5))
# bit-viewed as bf16 ~= exp(0.125*s), max rel err ~3.5%, rms ~1.8%,
# mean offset cancels in the softmax normalization.
SCH_SCALE = float(np.float32(0.125 * 128.0 / np.log(2.0)))
SCH_BIAS = float(np.float32(16256.0 - 6.0 + 0.5))
# kt indices whose exp tile runs on the Vector engine (Schraudolph); the
# rest use the Scalar engine's ACT spline exp. 7/16 on DVE balances the
# engines once the normalize chain (also DVE) is accounted for.
DVE_KT = frozenset((2, 4, 6, 8, 10, 12, 13))
AV_LAG = 3  # attnV matmuls trail their exp by 3 kts in the PE queue


def build_nc(S=2048, D=1024, heads=8, debug=False):
    """Build + compile the per-core Bass program. Dims parameterizable for
    small-scale simulation; defaults are the real shapes."""
    import concourse.bass as bass
    import concourse.mybir as mybir
    import concourse.tile as tile
    from concourse import bacc
    from concourse.alu_op_type import AluOpType

    f32 = mybir.dt.float32
    bf16 = mybir.dt.bfloat16
    i16 = mybir.dt.int16
    FT = mybir.ActivationFunctionType

    E = heads * HD              # per-core head channels (512)
    NDT = D // 128              # d-tiles (8)
    NST = S // 128              # s-tiles / k-tiles (16)
    NSC = S // 512              # 512-wide s-chunks (4)
    NET = E // 128              # e-tiles == head pairs (4)
    NQC = S // 512              # q-chunks (4)
    assert NQC % NET == 0 or NET % NQC == 0
    VW = 65                     # v columns per head incl. ones column
    dve_kt = set(k for k in DVE_KT if k < NST)

    nc = bacc.Bacc("TRN2", target_bir_lowering=False, debug=debug)

    xT_d = nc.dram_tensor("xT", [D, S], bf16, kind="ExternalInput")
    wq_d = nc.dram_tensor("wq", [D, E], bf16, kind="ExternalInput")
    wk_d = nc.dram_tensor("wk", [D, E], bf16, kind="ExternalInput")
    wv_d = nc.dram_tensor("wv", [D, E], bf16, kind="ExternalInput")
    wo_d = nc.dram_tensor("wo", [E, D], bf16, kind="ExternalInput")
    ones_d = nc.dram_tensor("ones64", [1, 64], bf16, kind="ExternalInput")
    out_d = nc.dram_tensor("out", [S, D], f32, kind="ExternalOutput")

    from contextlib import ExitStack

    with tile.TileContext(nc) as tc, ExitStack() as ctx:
        const = ctx.enter_context(tc.tile_pool(name="const", bufs=1))
        psp = ctx.enter_context(tc.tile_pool(name="psp", bufs=3, space="PSUM"))
        attn_ps = ctx.enter_context(tc.tile_pool(name="attn_ps", bufs=1, space="PSUM"))
        expp = ctx.enter_context(tc.tile_pool(name="expp", bufs=6))
        recipp = ctx.enter_context(tc.tile_pool(name="recipp", bufs=2))
        bcastp = ctx.enter_context(tc.tile_pool(name="bcastp", bufs=2))
        outst = ctx.enter_context(tc.tile_pool(name="outst", bufs=3))

        # ---- persistent SBUF tensors ----
        xT_sb = const.tile([128, NDT, S], bf16, tag="xT_sb")
        wq_sb = const.tile([128, NDT, E], bf16, tag="wq_sb")
        wk_sb = const.tile([128, NDT, E], bf16, tag="wk_sb")
        wv_sb = const.tile([128, NDT, E], bf16, tag="wv_sb")
        wo_sb = const.tile([128, NET, D], bf16, tag="wo_sb")
        ones64 = const.tile([1, 64], bf16, tag="ones64")
        qT = [const.tile([128, S], bf16, tag=f"qT{p}", name=f"qT{p}") for p in range(NET)]
        kT = [const.tile([128, S], bf16, tag=f"kT{p}", name=f"kT{p}") for p in range(NET)]
        vhat = [const.tile([128, heads, VW], bf16, tag=f"vh{st}", name=f"vh{st}") for st in range(NST)]
        attn_norm = [const.tile([128, S], bf16, tag=f"an{p}", name=f"an{p}") for p in range(NET)]

        # ---- input DMAs (xT chunked along S so stage1 starts early) ----
        nc.sync.dma_start(out=wv_sb, in_=wv_d.ap().rearrange("(t p) e -> p t e", p=128))
        xT_r = xT_d.ap().rearrange("(t p) s -> p t s", p=128)
        for c in range(NSC):
            sl = slice(c * 512, (c + 1) * 512)
            nc.sync.dma_start(out=xT_sb[:, :, sl], in_=xT_r[:, :, sl])
        nc.sync.dma_start(out=wq_sb, in_=wq_d.ap().rearrange("(t p) e -> p t e", p=128))
        nc.sync.dma_start(out=wk_sb, in_=wk_d.ap().rearrange("(t p) e -> p t e", p=128))
        nc.sync.dma_start(out=wo_sb, in_=wo_d.ap().rearrange("(t p) d -> p t d", p=128))
        nc.sync.dma_start(out=ones64, in_=ones_d.ap())

        # ---- stage 1: vhat = x @ wv (+ ones columns) ----
        for st in range(NST):
            nc.vector.memset(vhat[st], 1.0)
            ps = psp.tile([128, 1024], f32, tag="ps")
            for dt in range(NDT):
                nc.tensor.matmul(
                    ps[:, 0:E],
                    lhsT=xT_sb[:, dt, st * 128:(st + 1) * 128],
                    rhs=wv_sb[:, dt, :],
                    start=(dt == 0),
                    stop=(dt == NDT - 1),
                )
            nc.vector.tensor_copy(
                out=vhat[st][:, :, 0:HD],
                in_=ps[:, 0:E].rearrange("p (h c) -> p h c", c=HD),
            )

        # ---- stage 1: qT, kT = w.T @ xT (dt-outer: one LDWEIGHTS per 4 MMs) ----
        for p in range(NET):
            for w_sb, dstT in ((wq_sb, qT[p]), (wk_sb, kT[p])):
                ntile = (NSC + 1) // 2
                tiles = [psp.tile([128, 1024], f32, tag="ps", name=f"qk{p}_{0 if w_sb is wq_sb else 1}_{h}")
                         for h in range(ntile)]
                for dt in range(NDT):
                    for sc in range(NSC):
                        nc.tensor.matmul(
                            tiles[sc // 2][:, (sc % 2) * 512:(sc % 2) * 512 + 512],
                            lhsT=w_sb[:, dt, p * 128:(p + 1) * 128],
                            rhs=xT_sb[:, dt, sc * 512:(sc + 1) * 512],
                            start=(dt == 0),
                            stop=(dt == NDT - 1),
                            skip_group_check=True,
                        )
                for h in range(ntile):
                    w = min(1024, S - h * 1024)
                    nc.scalar.copy(out=dstT[:, h * 1024:h * 1024 + w], in_=tiles[h][:, 0:w])

        # ---- attention ----
        # The normalize chain of iteration i and the out-projection of q-chunk
        # qc are EMITTED inside later kt-streams (deferred) so the in-order PE
        # queue never head-of-line blocks on cross-engine normalize latency.
        DC = min(512, D)
        NDC = D // DC

        def make_norm(qc, p, aA, aB):
            state = {}

            def s0():
                state["dA"] = recipp.tile([1, 512], bf16, tag="dA", name=f"dA{qc}_{p}")
                state["dB"] = recipp.tile([1, 512], bf16, tag="dB", name=f"dB{qc}_{p}")
                nc.vector.tensor_copy(out=state["dA"], in_=aA[64:65, :])
                nc.vector.tensor_copy(out=state["dB"], in_=aB[64:65, :])

            def s1():
                bc = psp.tile([128, 1024], f32, tag="ps", name=f"bc{qc}_{p}")
                nc.tensor.matmul(bc[0:64, 0:512], lhsT=ones64, rhs=state["dA"],
                                 start=True, stop=True, skip_group_check=True)
                nc.tensor.matmul(bc[64:128, 0:512], lhsT=ones64, rhs=state["dB"],
                                 start=True, stop=True, skip_group_check=True)
                state["rbc"] = bcastp.tile([128, 512], f32, tag="rbc", name=f"rbc{qc}_{p}")
                nc.vector.reciprocal_approx_fast(out=state["rbc"], in_=bc[:, 0:512])

            def s2():
                qsl = slice(qc * 512, (qc + 1) * 512)
                nc.vector.scalar_tensor_tensor(
                    out=attn_norm[p][0:64, qsl], in0=aA[0:64, :], scalar=1.0,
                    in1=state["rbc"][0:64, :], op0=AluOpType.mult, op1=AluOpType.mult,
                )

            def s3():
                qsl = slice(qc * 512, (qc + 1) * 512)
                nmB = bcastp.tile([64, 512], bf16, tag="nmB")
                nc.vector.scalar_tensor_tensor(
                    out=nmB, in0=aB[0:64, :], scalar=1.0,
                    in1=state["rbc"][64:128, :], op0=AluOpType.mult, op1=AluOpType.mult,
                )
                nc.sync.dma_start(out=attn_norm[p][64:128, qsl], in_=nmB)

            return [s0, s1, s2, s3]

        def make_proj(st):
            def emit():
                ps = psp.tile([128, 1024], f32, tag="ps")
                for pp in range(NET):
                    for dc in range(NDC):
                        nc.tensor.matmul(
                            ps[:, dc * DC:(dc + 1) * DC],
                            lhsT=attn_norm[pp][:, st * 128:(st + 1) * 128],
                            rhs=wo_sb[:, pp, dc * DC:(dc + 1) * DC],
                            start=(pp == 0),
                            stop=(pp == NET - 1),
                            skip_group_check=True,
                        )
                ot = outst.tile([128, D], f32, tag="ot")
                nc.scalar.copy(out=ot[:, 0:NDC * DC], in_=ps[:, 0:NDC * DC])
                nc.sync.dma_start(
                    out=out_d.ap()[st * 128:(st + 1) * 128, :],
                    in_=ot[:, 0:D],
                )
            return emit

        pending_norm = []
        pending_proj = []
        for qc in range(NQC):
            for p in range(NET):
                hA, hB = 2 * p, 2 * p + 1
                aA = attn_ps.tile([VW, 512], f32, tag="attnA", name=f"aA{qc}_{p}")
                aB = attn_ps.tile([VW, 512], f32, tag="attnB", name=f"aB{qc}_{p}")

                def emit_av(kt, ex):
                    # attnV accumulation: one K=128 matmul per (kt, head)
                    nc.tensor.matmul(
                        aA, lhsT=vhat[kt][:, hA, :], rhs=ex[:, 0:512],
                        start=(kt == 0), stop=(kt == NST - 1), skip_group_check=True,
                    )
                    nc.tensor.matmul(
                        aB, lhsT=vhat[kt][:, hB, :], rhs=ex[:, 512:1024],
                        start=(kt == 0), stop=(kt == NST - 1), skip_group_check=True,
                    )

                av_fifo = []  # attnV trails by AV_LAG kts
                for kt in range(NST):
                    sc_ps = psp.tile([128, 1024], f32, tag="ps")
                    # scoresT for the head pair, row-packed (partitions 0-63 / 64-127)
                    nc.tensor.matmul(
                        sc_ps[:, 0:512],
                        lhsT=kT[p][0:HD, kt * 128:(kt + 1) * 128],
                        rhs=qT[p][0:HD, qc * 512:(qc + 1) * 512],
                        start=True, stop=True,
                    )
                    nc.tensor.matmul(
                        sc_ps[:, 512:1024],
                        lhsT=kT[p][64:64 + HD, kt * 128:(kt + 1) * 128],
                        rhs=qT[p][64:64 + HD, qc * 512:(qc + 1) * 512],
                        start=True, stop=True,
                    )
                    if pending_norm and kt >= 1:
                        pending_norm.pop(0)()
                    if len(av_fifo) >= AV_LAG:
                        emit_av(*av_fifo.pop(0))
                    ex = expp.tile([128, 1024], bf16, tag="exp")
                    if kt in dve_kt:
                        # Schraudolph bit-trick exp on the Vector engine
                        nc.vector.tensor_scalar(
                            out=ex.bitcast(i16), in0=sc_ps,
                            scalar1=SCH_SCALE, scalar2=SCH_BIAS,
                            op0=AluOpType.mult, op1=AluOpType.add,
                        )
                    else:
                        nc.scalar.activation(out=ex, in_=sc_ps, func=FT.Exp, scale=0.125)
                    av_fifo.append((kt, ex))
                    if kt == 5 and pending_proj:
                        pending_proj.pop(0)()
                for item in av_fifo:
                    emit_av(*item)
                pending_norm = make_norm(qc, p, aA, aB)
                pending_norm.pop(0)()  # denominator casts: DVE is free right now
            pending_proj.extend(
                make_proj(st)
                for st in range(qc * (NST // NQC), (qc + 1) * (NST // NQC))
            )
        # drain deferred work
        for step in pending_norm:
            step()
        for emit in pending_proj:
            emit()

    nc.compile()
    return nc


_NC_CACHE = {}


def _get_nc():
    if "nc" not in _NC_CACHE:
        _NC_CACHE["nc"] = build_nc()
    return _NC_CACHE["nc"]


def shard_inputs(x, w_qkv, w_out):
    """Host-side shard + layout prep. Returns in_maps for 8 cores."""
    D = D_FULL
    E = HEADS_PER_CORE * HD
    in_maps = []
    for core in range(N_CORES):
        b, g = core // 2, core % 2
        cs = slice(g * E, (g + 1) * E)
        in_maps.append({
            "xT": np.ascontiguousarray(x[b].T).astype(BF16),
            "wq": w_qkv[:, 0 * D:1 * D][:, cs].astype(BF16),
            "wk": w_qkv[:, 1 * D:2 * D][:, cs].astype(BF16),
            "wv": w_qkv[:, 2 * D:3 * D][:, cs].astype(BF16),
            "wo": w_out[cs, :].astype(BF16),
            "ones64": np.ones((1, 64), dtype=BF16),
        })
    return in_maps


def kernel(x, w_qkv, w_out):
    from concourse.bass_utils import run_bass_kernel_spmd

    x = np.asarray(x)
    w_qkv = np.asarray(w_qkv)
    w_out = np.asarray(w_out)
    nc = _get_nc()
    in_maps = shard_inputs(x, w_qkv, w_out)
    res = run_bass_kernel_spmd(nc, in_maps, list(range(N_CORES)))
    outs = [res.results[i]["out"] for i in range(N_CORES)]
    full = np.empty((B_FULL, S_FULL, D_FULL), np.float32)
    for b in range(B_FULL):
        full[b] = outs[2 * b] + outs[2 * b + 1]
    return full


# revision 18
# speedup vs baseline: 1.1760x; 1.1760x over previous
"""Trainium2 Bass kernel for nn_AttentionModel_88905823027207.

Full inputs:  x [4, 2048, 1024] f32, w_qkv [1024, 3072] f32, w_out [1024, 1024] f32
Full output:  [4, 2048, 1024] f32  (multi-head attention, 16 heads, + out proj)

Sharding: 8 cores = (batch b in 0..3) x (head-group g in 0..1).
Each core computes 8 heads of one batch element and the partial out-projection
for its head-group's rows of w_out; the host sums the two partials per batch.

Per-core kernel (all matmuls bf16 with fp32 PSUM accumulation):
  stage1: qT,kT [512, S] = w{q,k}.T @ x.T (dt-outer so each LDWEIGHTS serves
          4 matmuls); vhat [S, 8*65] = x @ wv with a ones-column per head
          (attnV then also accumulates the softmax denominator as row 64).
  attention (per head-pair, row-packed across PE row-groups via partition
          offsets 0/64): scoresT [k, q] -> exp(0.125*s) -> bf16, split
          between the Scalar engine (ACT spline exp) and the Vector engine
          (Schraudolph int16 bit-trick exp) so neither is the bottleneck
          -> attnV accumulation [65, 512] in PSUM (double-buffered).
  normalize: denom rows -> bf16 on Scalar, PE ones-matmul broadcast, DVE
          reciprocal_approx_fast, fused (PSUM evac x recip) multiplies.
  out-proj: attn_normT @ w_out rows, Scalar-engine evacuation, fp32 partial
          out to DRAM.

PSUM budget: one shared [128,1024] ring (bufs=2, 4 banks) carries scores /
stage1 accumulators / broadcast / out-proj; attnV pair is double-buffered
(4 banks). Total 8 banks.
"""

import numpy as np
import ml_dtypes

BF16 = ml_dtypes.bfloat16

# Full-problem dims (hardcoded per harness contract)
B_FULL, S_FULL, D_FULL, H_FULL, HD = 4, 2048, 1024, 16, 64
N_CORES = 8
HEADS_PER_CORE = H_FULL // 2  # 8

# Schraudolph bf16 exp constants: int16(trunc(s*0.125*184.665 + 16250.5))
# bit-viewed as bf16 ~= exp(0.125*s), max rel err ~3.5%, rms ~1.8%,
# mean offset cancels in the softmax normalization.
SCH_SCALE = float(np.float32(0.125 * 128.0 / np.log(2.0)))
SCH_BIAS = float(np.float32(16256.0 - 6.0 + 0.5))
# kt indices whose exp tile runs on the Vector engine (Schraudolph); the
# rest use the Scalar engine's ACT spline exp. 7/16 on DVE balances the
# engines once the normalize chain (also DVE) is accounted for.
DVE_KT = frozenset((2, 4, 6, 8, 10, 12, 13))
AV_LAG = 2  # attnV matmuls trail their exp by 2 kts in the PE queue


def build_nc(S=2048, D=1024, heads=8, debug=False):
    """Build + compile the per-core Bass program. Dims parameterizable for
    small-scale simulation; defaults are the real shapes."""
    import concourse.bass as bass
    import concourse.mybir as mybir
    import concourse.tile as tile
    from concourse import bacc
    from concourse.alu_op_type import AluOpType

    f32 = mybir.dt.float32
    bf16 = mybir.dt.bfloat16
    i16 = mybir.dt.int16
    FT = mybir.ActivationFunctionType

    E = heads * HD              # per-core head channels (512)
    NDT = D // 128              # d-tiles (8)
    NST = S // 128              # s-tiles / k-tiles (16)
    NSC = S // 512              # 512-wide s-chunks (4)
    NET = E // 128              # e-tiles == head pairs (4)
    NQC = S // 512              # q-chunks (4)
    assert NQC % NET == 0 or NET % NQC == 0
    VW = 65                     # v columns per head incl. ones column
    dve_kt = set(k for k in DVE_KT if k < NST)

    nc = bacc.Bacc("TRN2", target_bir_lowering=False, debug=debug)

    xT_d = nc.dram_tensor("xT", [D, S], bf16, kind="ExternalInput")
    wq_d = nc.dram_tensor("wq", [D, E], bf16, kind="ExternalInput")
    wk_d = nc.dram_tensor("wk", [D, E], bf16, kind="ExternalInput")
    wv_d = nc.dram_tensor("wv", [D, E], bf16, kind="ExternalInput")
    wo_d = nc.dram_tensor("wo", [E, D], bf16, kind="ExternalInput")
    ones_d = nc.dram_tensor("ones64", [1, 64], bf16, kind="ExternalInput")
    out_d = nc.dram_tensor("out", [S, D], f32, kind="ExternalOutput")

    from contextlib import ExitStack

    with tile.TileContext(nc) as tc, ExitStack() as ctx:
        const = ctx.enter_context(tc.tile_pool(name="const", bufs=1))
        psp = ctx.enter_context(tc.tile_pool(name="psp", bufs=3, space="PSUM"))
        attn_ps = ctx.enter_context(tc.tile_pool(name="attn_ps", bufs=1, space="PSUM"))
        expp = ctx.enter_context(tc.tile_pool(name="expp", bufs=6))
        recipp = ctx.enter_context(tc.tile_pool(name="recipp", bufs=2))
        bcastp = ctx.enter_context(tc.tile_pool(name="bcastp", bufs=2))
        outst = ctx.enter_context(tc.tile_pool(name="outst", bufs=3))

        # ---- persistent SBUF tensors ----
        xT_sb = const.tile([128, NDT, S], bf16, tag="xT_sb")
        wq_sb = const.tile([128, NDT, E], bf16, tag="wq_sb")
        wk_sb = const.tile([128, NDT, E], bf16, tag="wk_sb")
        wv_sb = const.tile([128, NDT, E], bf16, tag="wv_sb")
        wo_sb = const.tile([128, NET, D], bf16, tag="wo_sb")
        ones64 = const.tile([1, 64], bf16, tag="ones64")
        qT = [const.tile([128, S], bf16, tag=f"qT{p}", name=f"qT{p}") for p in range(NET)]
        kT = [const.tile([128, S], bf16, tag=f"kT{p}", name=f"kT{p}") for p in range(NET)]
        vhat = [const.tile([128, heads, VW], bf16, tag=f"vh{st}", name=f"vh{st}") for st in range(NST)]
        attn_norm = [const.tile([128, S], bf16, tag=f"an{p}", name=f"an{p}") for p in range(NET)]

        # ---- input DMAs (xT chunked along S so stage1 starts early) ----
        nc.scalar.dma_start(out=wv_sb, in_=wv_d.ap().rearrange("(t p) e -> p t e", p=128))
        xT_r = xT_d.ap().rearrange("(t p) s -> p t s", p=128)
        for c in range(NSC):
            sl = slice(c * 512, (c + 1) * 512)
            nc.sync.dma_start(out=xT_sb[:, :, sl], in_=xT_r[:, :, sl])
        nc.scalar.dma_start(out=wq_sb, in_=wq_d.ap().rearrange("(t p) e -> p t e", p=128))
        nc.scalar.dma_start(out=wk_sb, in_=wk_d.ap().rearrange("(t p) e -> p t e", p=128))
        nc.gpsimd.dma_start(out=wo_sb, in_=wo_d.ap().rearrange("(t p) d -> p t d", p=128))
        nc.scalar.dma_start(out=ones64, in_=ones_d.ap())

        # ---- stage 1: vhat = x @ wv (+ ones columns) ----
        for st in range(NST):
            nc.vector.memset(vhat[st], 1.0)
            ps = psp.tile([128, 1024], f32, tag="ps")
            for dt in range(NDT):
                nc.tensor.matmul(
                    ps[:, 0:E],
                    lhsT=xT_sb[:, dt, st * 128:(st + 1) * 128],
                    rhs=wv_sb[:, dt, :],
                    start=(dt == 0),
                    stop=(dt == NDT - 1),
                )
            nc.vector.tensor_copy(
                out=vhat[st][:, :, 0:HD],
                in_=ps[:, 0:E].rearrange("p (h c) -> p h c", c=HD),
            )

        # ---- stage 1: qT, kT = w.T @ xT (dt-outer: one LDWEIGHTS per 4 MMs) ----
        for p in range(NET):
            for w_sb, dstT in ((wq_sb, qT[p]), (wk_sb, kT[p])):
                ntile = (NSC + 1) // 2
                tiles = [psp.tile([128, 1024], f32, tag="ps", name=f"qk{p}_{0 if w_sb is wq_sb else 1}_{h}")
                         for h in range(ntile)]
                for dt in range(NDT):
                    for sc in range(NSC):
                        nc.tensor.matmul(
                            tiles[sc // 2][:, (sc % 2) * 512:(sc % 2) * 512 + 512],
                            lhsT=w_sb[:, dt, p * 128:(p + 1) * 128],
                            rhs=xT_sb[:, dt, sc * 512:(sc + 1) * 512],
                            start=(dt == 0),
                            stop=(dt == NDT - 1),
                            skip_group_check=True,
                        )
                for h in range(ntile):
                    w = min(1024, S - h * 1024)
                    nc.scalar.copy(out=dstT[:, h * 1024:h * 1024 + w], in_=tiles[h][:, 0:w])

        # ---- attention ----
        # The normalize chain of iteration i and the out-projection of q-chunk
        # qc are EMITTED inside later kt-streams (deferred) so the in-order PE
        # queue never head-of-line blocks on cross-engine normalize latency.
        DC = min(512, D)
        NDC = D // DC

        def make_norm(qc, p, aA, aB):
            state = {}

            def s0():
                state["dA"] = recipp.tile([1, 512], bf16, tag="dA", name=f"dA{qc}_{p}")
                state["dB"] = recipp.tile([1, 512], bf16, tag="dB", name=f"dB{qc}_{p}")
                nc.vector.tensor_copy(out=state["dA"], in_=aA[64:65, :])
                nc.vector.tensor_copy(out=state["dB"], in_=aB[64:65, :])

            def s1():
                bc = psp.tile([128, 1024], f32, tag="ps", name=f"bc{qc}_{p}")
                nc.tensor.matmul(bc[0:64, 0:512], lhsT=ones64, rhs=state["dA"],
                                 start=True, stop=True, skip_group_check=True)
                nc.tensor.matmul(bc[64:128, 0:512], lhsT=ones64, rhs=state["dB"],
                                 start=True, stop=True, skip_group_check=True)
                state["rbc"] = bcastp.tile([128, 512], f32, tag="rbc", name=f"rbc{qc}_{p}")
                nc.vector.reciprocal_approx_fast(out=state["rbc"], in_=bc[:, 0:512])

            def s2():
                qsl = slice(qc * 512, (qc + 1) * 512)
                nc.vector.scalar_tensor_tensor(
                    out=attn_norm[p][0:64, qsl], in0=aA[0:64, :], scalar=1.0,
                    in1=state["rbc"][0:64, :], op0=AluOpType.mult, op1=AluOpType.mult,
                )

            def s3():
                qsl = slice(qc * 512, (qc + 1) * 512)
                nmB = bcastp.tile([64, 512], bf16, tag="nmB")
                nc.vector.scalar_tensor_tensor(
                    out=nmB, in0=aB[0:64, :], scalar=1.0,
                    in1=state["rbc"][64:128, :], op0=AluOpType.mult, op1=AluOpType.mult,
                )
                nc.sync.dma_start(out=attn_norm[p][64:128, qsl], in_=nmB)

            return [s0, s1, s2, s3]

        def make_proj(st):
            def emit():
                ps = psp.tile([128, 1024], f32, tag="ps")
                for pp in range(NET):
                    for dc in range(NDC):
                        nc.tensor.matmul(
                            ps[:, dc * DC:(dc + 1) * DC],
                            lhsT=attn_norm[pp][:, st * 128:(st + 1) * 128],
                            rhs=wo_sb[:, pp, dc * DC:(dc + 1) * DC],
                            start=(pp == 0),
                            stop=(pp == NET - 1),
                            skip_group_check=True,
                        )
                ot = outst.tile([128, D], f32, tag="ot")
                nc.scalar.copy(out=ot[:, 0:NDC * DC], in_=ps[:, 0:NDC * DC])
                nc.sync.dma_start(
                    out=out_d.ap()[st * 128:(st + 1) * 128, :],
                    in_=ot[:, 0:D],
                )
            return emit

        pending_norm = []
        pending_proj = []
        for qc in range(NQC):
            for p in range(NET):
                hA, hB = 2 * p, 2 * p + 1
                aA = attn_ps.tile([VW, 512], f32, tag="attnA", name=f"aA{qc}_{p}")
                aB = attn_ps.tile([VW, 512], f32, tag="attnB", name=f"aB{qc}_{p}")

                def emit_av(kt, ex):
                    # attnV accumulation: one K=128 matmul per (kt, head)
                    nc.tensor.matmul(
                        aA, lhsT=vhat[kt][:, hA, :], rhs=ex[:, 0:512],
                        start=(kt == 0), stop=(kt == NST - 1), skip_group_check=True,
                    )
                    nc.tensor.matmul(
                        aB, lhsT=vhat[kt][:, hB, :], rhs=ex[:, 512:1024],
                        start=(kt == 0), stop=(kt == NST - 1), skip_group_check=True,
                    )

                av_fifo = []  # attnV trails by AV_LAG kts
                for kt in range(NST):
                    sc_ps = psp.tile([128, 1024], f32, tag="ps")
                    # scoresT for the head pair, row-packed (partitions 0-63 / 64-127)
                    nc.tensor.matmul(
                        sc_ps[:, 0:512],
                        lhsT=kT[p][0:HD, kt * 128:(kt + 1) * 128],
                        rhs=qT[p][0:HD, qc * 512:(qc + 1) * 512],
                        start=True, stop=True,
                    )
                    nc.tensor.matmul(
                        sc_ps[:, 512:1024],
                        lhsT=kT[p][64:64 + HD, kt * 128:(kt + 1) * 128],
                        rhs=qT[p][64:64 + HD, qc * 512:(qc + 1) * 512],
                        start=True, stop=True,
                    )
                    if pending_norm and kt >= 1:
                        pending_norm.pop(0)()
                    if len(av_fifo) >= AV_LAG:
                        emit_av(*av_fifo.pop(0))
                    ex = expp.tile([128, 1024], bf16, tag="exp")
                    if kt in dve_kt:
                        # Schraudolph bit-trick exp on the Vector engine
                        nc.vector.tensor_scalar(
                            out=ex.bitcast(i16), in0=sc_ps,
                            scalar1=SCH_SCALE, scalar2=SCH_BIAS,
                            op0=AluOpType.mult, op1=AluOpType.add,
                        )
                    else:
                        nc.scalar.activation(out=ex, in_=sc_ps, func=FT.Exp, scale=0.125)
                    av_fifo.append((kt, ex))
                    if kt == 5 and pending_proj:
                        pending_proj.pop(0)()
                for item in av_fifo:
                    emit_av(*item)
                pending_norm = make_norm(qc, p, aA, aB)
                pending_norm.pop(0)()  # denominator casts: DVE is free right now
            pending_proj.extend(
                make_proj(st)
                for st in range(qc * (NST // NQC), (qc + 1) * (NST // NQC))
            )
        # drain deferred work
        for step in pending_norm:
            step()
        for emit in pending_proj:
            emit()

    nc.compile()
    return nc


_NC_CACHE = {}


def _get_nc():
    if "nc" not in _NC_CACHE:
        _NC_CACHE["nc"] = build_nc()
    return _NC_CACHE["nc"]


def shard_inputs(x, w_qkv, w_out):
    """Host-side shard + layout prep. Returns in_maps for 8 cores."""
    D = D_FULL
    E = HEADS_PER_CORE * HD
    in_maps = []
    for core in range(N_CORES):
        b, g = core // 2, core % 2
        cs = slice(g * E, (g + 1) * E)
        in_maps.append({
            "xT": np.ascontiguousarray(x[b].T).astype(BF16),
            "wq": w_qkv[:, 0 * D:1 * D][:, cs].astype(BF16),
            "wk": w_qkv[:, 1 * D:2 * D][:, cs].astype(BF16),
            "wv": w_qkv[:, 2 * D:3 * D][:, cs].astype(BF16),
            "wo": w_out[cs, :].astype(BF16),
            "ones64": np.ones((1, 64), dtype=BF16),
        })
    return in_maps


def kernel(x, w_qkv, w_out):
    from concourse.bass_utils import run_bass_kernel_spmd

    x = np.asarray(x)
    w_qkv = np.asarray(w_qkv)
    w_out = np.asarray(w_out)
    nc = _get_nc()
    in_maps = shard_inputs(x, w_qkv, w_out)
    res = run_bass_kernel_spmd(nc, in_maps, list(range(N_CORES)))
    outs = [res.results[i]["out"] for i in range(N_CORES)]
    full = np.empty((B_FULL, S_FULL, D_FULL), np.float32)
    for b in range(B_FULL):
        full[b] = outs[2 * b] + outs[2 * b + 1]
    return full


# revision 19
# speedup vs baseline: 1.2001x; 1.0204x over previous
"""Trainium2 Bass kernel for nn_AttentionModel_88905823027207.

Full inputs:  x [4, 2048, 1024] f32, w_qkv [1024, 3072] f32, w_out [1024, 1024] f32
Full output:  [4, 2048, 1024] f32  (multi-head attention, 16 heads, + out proj)

Sharding: 8 cores = (batch b in 0..3) x (head-group g in 0..1).
Each core computes 8 heads of one batch element and the partial out-projection
for its head-group's rows of w_out; the host sums the two partials per batch.

Per-core kernel (all matmuls bf16 with fp32 PSUM accumulation):
  stage1: qT,kT [512, S] = w{q,k}.T @ x.T (dt-outer so each LDWEIGHTS serves
          4 matmuls); vhat [S, 8*65] = x @ wv with a ones-column per head
          (attnV then also accumulates the softmax denominator as row 64).
  attention (per head-pair, row-packed across PE row-groups via partition
          offsets 0/64): scoresT [k, q] -> exp(0.125*s) -> bf16, split
          between the Scalar engine (ACT spline exp) and the Vector engine
          (Schraudolph int16 bit-trick exp) so neither is the bottleneck
          -> attnV accumulation [65, 512] in PSUM (double-buffered).
  normalize: denom rows -> bf16 on Scalar, PE ones-matmul broadcast, DVE
          reciprocal_approx_fast, fused (PSUM evac x recip) multiplies.
  out-proj: attn_normT @ w_out rows, Scalar-engine evacuation, fp32 partial
          out to DRAM.

PSUM budget: one shared [128,1024] ring (bufs=2, 4 banks) carries scores /
stage1 accumulators / broadcast / out-proj; attnV pair is double-buffered
(4 banks). Total 8 banks.
"""

import numpy as np
import ml_dtypes

BF16 = ml_dtypes.bfloat16

# Full-problem dims (hardcoded per harness contract)
B_FULL, S_FULL, D_FULL, H_FULL, HD = 4, 2048, 1024, 16, 64
N_CORES = 8
HEADS_PER_CORE = H_FULL // 2  # 8

# Schraudolph bf16 exp constants: int16(trunc(s*0.125*184.665 + 16250.5))
# bit-viewed as bf16 ~= exp(0.125*s), max rel err ~3.5%, rms ~1.8%,
# mean offset cancels in the softmax normalization.
SCH_SCALE = float(np.float32(0.125 * 128.0 / np.log(2.0)))
SCH_BIAS = float(np.float32(16256.0 - 6.0 + 0.5))
# kt indices whose exp tile runs on the Vector engine (Schraudolph); the
# rest use the Scalar engine's ACT spline exp. 7/16 on DVE balances the
# engines once the normalize chain (also DVE) is accounted for.
DVE_KT = frozenset((2, 4, 6, 8, 10, 12, 13))
AV_LAG = 3  # attnV matmuls trail their exp by 3 kts in the PE queue


def build_nc(S=2048, D=1024, heads=8, debug=False):
    """Build + compile the per-core Bass program. Dims parameterizable for
    small-scale simulation; defaults are the real shapes."""
    import concourse.bass as bass
    import concourse.mybir as mybir
    import concourse.tile as tile
    from concourse import bacc
    from concourse.alu_op_type import AluOpType

    f32 = mybir.dt.float32
    bf16 = mybir.dt.bfloat16
    i16 = mybir.dt.int16
    FT = mybir.ActivationFunctionType

    E = heads * HD              # per-core head channels (512)
    NDT = D // 128              # d-tiles (8)
    NST = S // 128              # s-tiles / k-tiles (16)
    NSC = S // 512              # 512-wide s-chunks (4)
    NET = E // 128              # e-tiles == head pairs (4)
    NQC = S // 512              # q-chunks (4)
    assert NQC % NET == 0 or NET % NQC == 0
    VW = 65                     # v columns per head incl. ones column
    dve_kt = set(k for k in DVE_KT if k < NST)

    nc = bacc.Bacc("TRN2", target_bir_lowering=False, debug=debug)

    xT_d = nc.dram_tensor("xT", [D, S], bf16, kind="ExternalInput")
    wq_d = nc.dram_tensor("wq", [D, E], bf16, kind="ExternalInput")
    wk_d = nc.dram_tensor("wk", [D, E], bf16, kind="ExternalInput")
    wv_d = nc.dram_tensor("wv", [D, E], bf16, kind="ExternalInput")
    wo_d = nc.dram_tensor("wo", [E, D], bf16, kind="ExternalInput")
    ones_d = nc.dram_tensor("ones64", [1, 64], bf16, kind="ExternalInput")
    out_d = nc.dram_tensor("out", [S, D], f32, kind="ExternalOutput")

    from contextlib import ExitStack

    with tile.TileContext(nc) as tc, ExitStack() as ctx:
        const = ctx.enter_context(tc.tile_pool(name="const", bufs=1))
        psp = ctx.enter_context(tc.tile_pool(name="psp", bufs=3, space="PSUM"))
        attn_ps = ctx.enter_context(tc.tile_pool(name="attn_ps", bufs=1, space="PSUM"))
        expp = ctx.enter_context(tc.tile_pool(name="expp", bufs=6))
        recipp = ctx.enter_context(tc.tile_pool(name="recipp", bufs=2))
        bcastp = ctx.enter_context(tc.tile_pool(name="bcastp", bufs=2))
        outst = ctx.enter_context(tc.tile_pool(name="outst", bufs=3))

        # ---- persistent SBUF tensors ----
        xT_sb = const.tile([128, NDT, S], bf16, tag="xT_sb")
        wq_sb = const.tile([128, NDT, E], bf16, tag="wq_sb")
        wk_sb = const.tile([128, NDT, E], bf16, tag="wk_sb")
        wv_sb = const.tile([128, NDT, E], bf16, tag="wv_sb")
        wo_sb = const.tile([128, NET, D], bf16, tag="wo_sb")
        ones64 = const.tile([1, 64], bf16, tag="ones64")
        qT = [const.tile([128, S], bf16, tag=f"qT{p}", name=f"qT{p}") for p in range(NET)]
        kT = [const.tile([128, S], bf16, tag=f"kT{p}", name=f"kT{p}") for p in range(NET)]
        vhat = [const.tile([128, heads, VW], bf16, tag=f"vh{st}", name=f"vh{st}") for st in range(NST)]
        attn_norm = [const.tile([128, S], bf16, tag=f"an{p}", name=f"an{p}") for p in range(NET)]

        # ---- input DMAs (xT chunked along S so stage1 starts early) ----
        nc.sync.dma_start(out=wv_sb, in_=wv_d.ap().rearrange("(t p) e -> p t e", p=128))
        xT_r = xT_d.ap().rearrange("(t p) s -> p t s", p=128)
        for c in range(NSC):
            sl = slice(c * 512, (c + 1) * 512)
            nc.sync.dma_start(out=xT_sb[:, :, sl], in_=xT_r[:, :, sl])
        nc.sync.dma_start(out=wq_sb, in_=wq_d.ap().rearrange("(t p) e -> p t e", p=128))
        nc.sync.dma_start(out=wk_sb, in_=wk_d.ap().rearrange("(t p) e -> p t e", p=128))
        nc.sync.dma_start(out=wo_sb, in_=wo_d.ap().rearrange("(t p) d -> p t d", p=128))
        nc.sync.dma_start(out=ones64, in_=ones_d.ap())

        # ---- stage 1: vhat = x @ wv (+ ones columns) ----
        for st in range(NST):
            nc.vector.memset(vhat[st], 1.0)
            ps = psp.tile([128, 1024], f32, tag="ps")
            for dt in range(NDT):
                nc.tensor.matmul(
                    ps[:, 0:E],
                    lhsT=xT_sb[:, dt, st * 128:(st + 1) * 128],
                    rhs=wv_sb[:, dt, :],
                    start=(dt == 0),
                    stop=(dt == NDT - 1),
                )
            nc.vector.tensor_copy(
                out=vhat[st][:, :, 0:HD],
                in_=ps[:, 0:E].rearrange("p (h c) -> p h c", c=HD),
            )

        # ---- stage 1: qT, kT = w.T @ xT (dt-outer: one LDWEIGHTS per 4 MMs) ----
        for p in range(NET):
            for w_sb, dstT in ((wq_sb, qT[p]), (wk_sb, kT[p])):
                ntile = (NSC + 1) // 2
                tiles = [psp.tile([128, 1024], f32, tag="ps", name=f"qk{p}_{0 if w_sb is wq_sb else 1}_{h}")
                         for h in range(ntile)]
                for dt in range(NDT):
                    for sc in range(NSC):
                        nc.tensor.matmul(
                            tiles[sc // 2][:, (sc % 2) * 512:(sc % 2) * 512 + 512],
                            lhsT=w_sb[:, dt, p * 128:(p + 1) * 128],
                            rhs=xT_sb[:, dt, sc * 512:(sc + 1) * 512],
                            start=(dt == 0),
                            stop=(dt == NDT - 1),
                            skip_group_check=True,
                        )
                for h in range(ntile):
                    w = min(1024, S - h * 1024)
                    nc.scalar.copy(out=dstT[:, h * 1024:h * 1024 + w], in_=tiles[h][:, 0:w])

        # ---- attention ----
        # The normalize chain of iteration i and the out-projection of q-chunk
        # qc are EMITTED inside later kt-streams (deferred) so the in-order PE
        # queue never head-of-line blocks on cross-engine normalize latency.
        DC = min(512, D)
        NDC = D // DC

        def make_norm(qc, p, aA, aB):
            state = {}

            def s0():
                state["dA"] = recipp.tile([1, 512], bf16, tag="dA", name=f"dA{qc}_{p}")
                state["dB"] = recipp.tile([1, 512], bf16, tag="dB", name=f"dB{qc}_{p}")
                nc.vector.tensor_copy(out=state["dA"], in_=aA[64:65, :])
                nc.vector.tensor_copy(out=state["dB"], in_=aB[64:65, :])

            def s1():
                bc = psp.tile([128, 1024], f32, tag="ps", name=f"bc{qc}_{p}")
                nc.tensor.matmul(bc[0:64, 0:512], lhsT=ones64, rhs=state["dA"],
                                 start=True, stop=True, skip_group_check=True)
                nc.tensor.matmul(bc[64:128, 0:512], lhsT=ones64, rhs=state["dB"],
                                 start=True, stop=True, skip_group_check=True)
                state["rbc"] = bcastp.tile([128, 512], f32, tag="rbc", name=f"rbc{qc}_{p}")
                nc.vector.reciprocal_approx_fast(out=state["rbc"], in_=bc[:, 0:512])

            def s2():
                qsl = slice(qc * 512, (qc + 1) * 512)
                nc.vector.scalar_tensor_tensor(
                    out=attn_norm[p][0:64, qsl], in0=aA[0:64, :], scalar=1.0,
                    in1=state["rbc"][0:64, :], op0=AluOpType.mult, op1=AluOpType.mult,
                )

            def s3():
                qsl = slice(qc * 512, (qc + 1) * 512)
                nmB = bcastp.tile([64, 512], bf16, tag="nmB")
                nc.vector.scalar_tensor_tensor(
                    out=nmB, in0=aB[0:64, :], scalar=1.0,
                    in1=state["rbc"][64:128, :], op0=AluOpType.mult, op1=AluOpType.mult,
                )
                nc.sync.dma_start(out=attn_norm[p][64:128, qsl], in_=nmB)

            return [s0, s1, s2, s3]

        def make_proj(st):
            def emit():
                ps = psp.tile([128, 1024], f32, tag="ps")
                for pp in range(NET):
                    for dc in range(NDC):
                        nc.tensor.matmul(
                            ps[:, dc * DC:(dc + 1) * DC],
                            lhsT=attn_norm[pp][:, st * 128:(st + 1) * 128],
                            rhs=wo_sb[:, pp, dc * DC:(dc + 1) * DC],
                            start=(pp == 0),
                            stop=(pp == NET - 1),
                            skip_group_check=True,
                        )
                ot = outst.tile([128, D], f32, tag="ot")
                nc.scalar.copy(out=ot[:, 0:NDC * DC], in_=ps[:, 0:NDC * DC])
                nc.sync.dma_start(
                    out=out_d.ap()[st * 128:(st + 1) * 128, :],
                    in_=ot[:, 0:D],
                )
            return emit

        pending_norm = []
        pending_proj = []
        for qc in range(NQC):
            for p in range(NET):
                hA, hB = 2 * p, 2 * p + 1
                aA = attn_ps.tile([VW, 512], f32, tag="attnA", name=f"aA{qc}_{p}")
                aB = attn_ps.tile([VW, 512], f32, tag="attnB", name=f"aB{qc}_{p}")

                def emit_av(kt, ex):
                    # attnV accumulation: one K=128 matmul per (kt, head)
                    nc.tensor.matmul(
                        aA, lhsT=vhat[kt][:, hA, :], rhs=ex[:, 0:512],
                        start=(kt == 0), stop=(kt == NST - 1), skip_group_check=True,
                    )
                    nc.tensor.matmul(
                        aB, lhsT=vhat[kt][:, hB, :], rhs=ex[:, 512:1024],
                        start=(kt == 0), stop=(kt == NST - 1), skip_group_check=True,
                    )

                av_fifo = []  # attnV trails by AV_LAG kts
                for kt in range(NST):
                    sc_ps = psp.tile([128, 1024], f32, tag="ps")
                    # scoresT for the head pair, row-packed (partitions 0-63 / 64-127)
                    nc.tensor.matmul(
                        sc_ps[:, 0:512],
                        lhsT=kT[p][0:HD, kt * 128:(kt + 1) * 128],
                        rhs=qT[p][0:HD, qc * 512:(qc + 1) * 512],
                        start=True, stop=True,
                    )
                    nc.tensor.matmul(
                        sc_ps[:, 512:1024],
                        lhsT=kT[p][64:64 + HD, kt * 128:(kt + 1) * 128],
                        rhs=qT[p][64:64 + HD, qc * 512:(qc + 1) * 512],
                        start=True, stop=True,
                    )
                    if pending_norm and kt >= 1:
                        pending_norm.pop(0)()
                    if len(av_fifo) >= AV_LAG:
                        emit_av(*av_fifo.pop(0))
                    ex = expp.tile([128, 1024], bf16, tag="exp")
                    if kt in dve_kt:
                        # Schraudolph bit-trick exp on the Vector engine
                        nc.vector.tensor_scalar(
                            out=ex.bitcast(i16), in0=sc_ps,
                            scalar1=SCH_SCALE, scalar2=SCH_BIAS,
                            op0=AluOpType.mult, op1=AluOpType.add,
                        )
                    else:
                        nc.scalar.activation(out=ex, in_=sc_ps, func=FT.Exp, scale=0.125)
                    av_fifo.append((kt, ex))
                    if kt == 5 and pending_proj:
                        pending_proj.pop(0)()
                for item in av_fifo:
                    emit_av(*item)
                pending_norm = make_norm(qc, p, aA, aB)
                pending_norm.pop(0)()  # denominator casts: DVE is free right now
            pending_proj.extend(
                make_proj(st)
                for st in range(qc * (NST // NQC), (qc + 1) * (NST // NQC))
            )
        # drain deferred work
        for step in pending_norm:
            step()
        for emit in pending_proj:
            emit()

    nc.compile()
    return nc


_NC_CACHE = {}


def _get_nc():
    if "nc" not in _NC_CACHE:
        _NC_CACHE["nc"] = build_nc()
    return _NC_CACHE["nc"]


def shard_inputs(x, w_qkv, w_out):
    """Host-side shard + layout prep. Returns in_maps for 8 cores."""
    D = D_FULL
    E = HEADS_PER_CORE * HD
    in_maps = []
    for core in range(N_CORES):
        b, g = core // 2, core % 2
        cs = slice(g * E, (g + 1) * E)
        in_maps.append({
            "xT": np.ascontiguousarray(x[b].T).astype(BF16),
            "wq": w_qkv[:, 0 * D:1 * D][:, cs].astype(BF16),
            "wk": w_qkv[:, 1 * D:2 * D][:, cs].astype(BF16),
            "wv": w_qkv[:, 2 * D:3 * D][:, cs].astype(BF16),
            "wo": w_out[cs, :].astype(BF16),
            "ones64": np.ones((1, 64), dtype=BF16),
        })
    return in_maps


def kernel(x, w_qkv, w_out):
    from concourse.bass_utils import run_bass_kernel_spmd

    x = np.asarray(x)
    w_qkv = np.asarray(w_qkv)
    w_out = np.asarray(w_out)
    nc = _get_nc()
    in_maps = shard_inputs(x, w_qkv, w_out)
    res = run_bass_kernel_spmd(nc, in_maps, list(range(N_CORES)))
    outs = [res.results[i]["out"] for i in range(N_CORES)]
    full = np.empty((B_FULL, S_FULL, D_FULL), np.float32)
    for b in range(B_FULL):
        full[b] = outs[2 * b] + outs[2 * b + 1]
    return full
